# revision 1
# baseline (speedup 1.0000x reference)
"""DarkChannel kernel for Trainium2: channel-min + 15x15 separable min-pool.

Full input img [16, 3, 1024, 1024] f32 -> output [16, 1, 1024, 1024] f32.
Batch-sharded across 8 NeuronCores (2 images per core), exact f32.

Per-core pipeline, per row-block (114 output rows, 9 blocks, both images
batched side-by-side in the free dimension at a 32B-aligned pitch):
  - load 128 input rows (7-row halo; edge rows clamp-replicated via
    broadcast DMAs) x 3 channels x 2 images
  - channel min: 2 DVE tensor_tensor min (second one in-place)
  - horizontal window-15 min via van Herk: two segmented min-scans
    (tensor_tensor_scan, op0=add with a +inf mask resets segments of 15;
    segment phase is defined purely by the masks so the image pitch can
    stay 32B-aligned) + one combine. Scan outputs are written displaced
    (+1 / +7) so the combine reads at 32B-aligned offsets.
  - vertical window-15 min via doubling tree: row shifts as shift-matrix
    matmuls on the tensor engine (PSUM), min on DVE (all-f32 PSUM reads)
"""
import sys
sys.path.insert(0, '/opt/trn_rl_repo')

import numpy as np

import concourse.bacc as bacc_mod
import concourse.mybir as mybir
from concourse.tile import TileContext
from concourse import bass_utils

F32 = mybir.dt.float32
BF16 = mybir.dt.bfloat16
MIN = mybir.AluOpType.min
ADD = mybir.AluOpType.add
INF = float('inf')

H = 1024
W = 1024
C = 3
NIMG = 2              # images per core
N_CORES = 8
RBLK = 114            # output rows per block
NBLK = 9
LPAD = 8              # left pad (32B aligned), >= 7
IP = 1040             # per-image pitch in padded arrays (mult of 8)
SEG = 15
XPW = NIMG * IP       # 2112
PSW = 2096            # scan output tiles (displaced writes)

_cache = {}


def _tt(nc, out, in0, in1, op=MIN):
    eng = nc.vector
    return eng.add_instruction(mybir.InstTensorTensor(
        name=nc.get_next_instruction_name(), op=op,
        ins=[eng.lower_ap(in0), eng.lower_ap(in1)],
        outs=[eng.lower_ap(out)]))


def _build():
    nc = bacc_mod.Bacc("TRN2", target_bir_lowering=False, debug=False,
                       num_devices=N_CORES)
    img = nc.dram_tensor("img", [NIMG, C, H, W], F32, kind="ExternalInput")
    out = nc.dram_tensor("out", [NIMG, 1, H, W], F32, kind="ExternalOutput")

    with TileContext(nc) as tc:
        with tc.tile_pool(name="const", bufs=1) as cpool, \
             tc.tile_pool(name="chin", bufs=3) as chpool, \
             tc.tile_pool(name="work", bufs=3) as wpool, \
             tc.tile_pool(name="vwork", bufs=2) as vpool, \
             tc.tile_pool(name="psum", bufs=2, space="PSUM") as ppool:

            # ---- constants ----
            mask_f = cpool.tile([128, XPW], F32)
            mask_b = cpool.tile([128, XPW], F32)
            nc.gpsimd.memset(mask_f[:], 0.0)
            nc.gpsimd.memset(mask_b[:], 0.0)
            for i in range(NIMG):
                nc.gpsimd.memset(mask_f[:, i * IP:i * IP + IP:SEG], INF)
                nc.gpsimd.memset(mask_b[:, i * IP + SEG - 1:i * IP + IP:SEG],
                                 INF)

            wmats = {}
            for d in (1, 2, 4, 7):
                wm = cpool.tile([128, 128], F32, tag=f"wm{d}")
                nc.gpsimd.memset(wm[:], 1.0)
                # lhsT W[k, m] = 1 iff m == k - d  => out[m] = in[m + d]
                nc.gpsimd.affine_select(
                    out=wm[:], in_=wm[:],
                    compare_op=mybir.AluOpType.is_equal, fill=0.0,
                    base=d, channel_multiplier=-1, pattern=[[1, 128]])
                wmats[d] = wm

            # ---- per-block pipeline (loads emitted 2 blocks ahead) ----
            def emit_loads(b):
                r0 = b * RBLK
                rout = min(RBLK, H - r0)
                lo = r0 - 7
                hi = r0 + rout + 7
                npart = hi - lo
                ct = chpool.tile([128, C * NIMG * IP], F32, tag="ch")
                ctv = ct[:].rearrange("p (c i w) -> p c i w", c=C, i=NIMG)
                for c in range(C):
                    for i in range(NIMG):
                        dma = nc.sync if (c * NIMG + i) % 2 == 0 else nc.scalar
                        dst0 = 0
                        src_lo = lo
                        if lo < 0:
                            dma.dma_start(
                                out=ctv[0:-lo, c, i, LPAD:LPAD + W],
                                in_=img[i, c, 0:1, :].to_broadcast((-lo, W)))
                            dst0 = -lo
                            src_lo = 0
                        src_hi = min(hi, H)
                        dma.dma_start(
                            out=ctv[dst0:dst0 + (src_hi - src_lo), c, i,
                                    LPAD:LPAD + W],
                            in_=img[i, c, src_lo:src_hi, :])
                        if hi > H:
                            dma.dma_start(
                                out=ctv[npart - (hi - H):npart, c, i,
                                        LPAD:LPAD + W],
                                in_=img[i, c, H - 1:H, :].to_broadcast(
                                    (hi - H, W)))
                return ctv

            pending = [emit_loads(0), emit_loads(1)]
            for b in range(NBLK):
                r0 = b * RBLK
                rout = min(RBLK, H - r0)
                ctv = pending.pop(0)
                if b + 2 < NBLK:
                    pending.append(emit_loads(b + 2))

                # channel min into padded xp
                xp = wpool.tile([128, XPW], F32, tag="xp")
                xpv = xp[:].rearrange("p (i w) -> p i w", i=NIMG)
                nc.gpsimd.memset(xpv[:, :, 0:LPAD], INF)
                nc.gpsimd.memset(xpv[:, :, LPAD + W:IP], INF)
                mid = xpv[:, :, LPAD:LPAD + W]
                _tt(nc, mid, ctv[:, 0, :, LPAD:LPAD + W],
                    ctv[:, 1, :, LPAD:LPAD + W])
                _tt(nc, mid, mid, ctv[:, 2, :, LPAD:LPAD + W])

                # h-pass: segmented van Herk scans, displaced writes
                ps = wpool.tile([128, PSW], F32, tag="ps")
                ss = wpool.tile([128, PSW], F32, tag="ss")
                nc.vector.tensor_tensor_scan(
                    out=ps[:, 1:1 + XPW], data0=mask_f[:], data1=xp[:],
                    initial=0.0, op0=ADD, op1=MIN)
                nc.vector.tensor_tensor_scan(
                    out=ss[:, 7:7 + XPW][:, ::-1], data0=mask_b[:, ::-1],
                    data1=xp[:, ::-1], initial=0.0, op0=ADD, op1=MIN)
                # combine: out col j = min(ss[j+1], ps[j+15]) (displaced:
                # ss at +8, ps at +16 -> both 32B aligned)
                hmin = vpool.tile([128, NIMG * W], F32, tag="hmin")
                hminv = hmin[:].rearrange("p (i w) -> p i w", i=NIMG)
                ssv = ss[:, 8:8 + NIMG * IP].rearrange(
                    "p (i w) -> p i w", i=NIMG)[:, :, 0:W]
                psv = ps[:, 16:16 + NIMG * IP].rearrange(
                    "p (i w) -> p i w", i=NIMG)[:, :, 0:W]
                _tt(nc, hminv[:], ssv, psv)

                # v-pass: PE shift matmuls + DVE min (all f32),
                # ping-ponged per image so DVE mins one image while PE
                # shifts the other
                cur = hmin
                for d in (1, 2, 4, 7):
                    for i in range(NIMG):
                        sh = ppool.tile([128, W], F32, tag=f"sh{i}")
                        for half in range(2):
                            nc.tensor.matmul(
                                sh[:, half * 512:(half + 1) * 512],
                                wmats[d][:],
                                cur[:, i * W + half * 512:
                                    i * W + (half + 1) * 512],
                                start=True, stop=True)
                        _tt(nc, cur[:, i * W:(i + 1) * W],
                            cur[:, i * W:(i + 1) * W], sh[:])

                for i in range(NIMG):
                    dma = nc.sync if i == 0 else nc.scalar
                    dma.dma_start(out=out[i, 0, r0:r0 + rout, :],
                                  in_=cur[0:rout, i * W:i * W + W])

    nc.compile()
    return nc


def kernel(img: np.ndarray) -> np.ndarray:
    assert img.shape == (16, 3, 1024, 1024) and img.dtype == np.float32
    if "nc" not in _cache:
        _cache["nc"] = _build()
    nc = _cache["nc"]
    in_maps = [{"img": np.ascontiguousarray(img[2 * k:2 * k + 2])}
               for k in range(N_CORES)]
    res = bass_utils.run_bass_kernel_spmd(
        nc, in_maps, core_ids=list(range(N_CORES)))
    return np.concatenate([r["out"] for r in res.results], axis=0)



# revision 8
# speedup vs baseline: 1.0671x; 1.0671x over previous
"""DarkChannel kernel for Trainium2: channel-min + 15x15 separable min-pool.

Full input img [16, 3, 1024, 1024] f32 -> output [16, 1, 1024, 1024] f32.
Batch-sharded across 8 NeuronCores (2 images per core).

Internals run in bf16 (tolerance 2e-2 >> bf16 rounding 2^-9), which:
  - halves SBUF traffic and enables the DVE 2x_1p mode
  - quarters PE time for the vertical shift matmuls vs f32
Input loads are gpsimd (SWDGE) DMAs casting f32->bf16 in the DMA datapath,
so no engine cycles are spent on dtype conversion. Output is written bf16
and upconverted on the host.

Per-core pipeline, per row-block (114 output rows, 9 blocks, 2 images
side-by-side in the free dim):
  - load 114 fresh input rows x 3 ch x 2 img (bf16-cast); the 14-row
    halo is a cheap SBUF->SBUF copy of the previous block's channel-min
    rows instead of an HBM re-read; edge blocks fill a large finite value
  - channel min: 2 DVE tensor_tensor mins (bf16 2x), writing padded A
  - horizontal window-15 min: 4-step doubling tree on DVE (shifts are
    free-dim AP offsets; TENSOR_TENSOR is DVE-only on this compiler)
  - vertical window-15 min: doubling tree; row shifts via shift-matrix
    matmuls on PE (bf16 -> PSUM f32); ACT copies each shift to SBUF
    bf16; DVE does the min at 2x. (Pool cannot read PSUM on TRN2.)
"""
import sys
sys.path.insert(0, '/opt/trn_rl_repo')

import numpy as np

import concourse.bacc as bacc_mod
import concourse.mybir as mybir
from concourse.tile import TileContext
from concourse import bass_utils

F32 = mybir.dt.float32
BF16 = mybir.dt.bfloat16
MIN = mybir.AluOpType.min
BIG = 3e38   # large finite; +inf would NaN-poison the shift matmuls (0*inf)

H = 1024
W = 1024
C = 3
NIMG = 2              # images per core
N_CORES = 8
RBLK = 114            # output rows per block
NBLK = 9
LPAD = 8              # left pad; image at [8, 1032)
IPH = 1056            # per-image pitch (2112 B, 32B aligned)
HALO = 14

_cache = {}


def _tt(eng, out, in0, in1, op=MIN):
    return eng.add_instruction(mybir.InstTensorTensor(
        name=eng.bass.get_next_instruction_name(), op=op,
        ins=[eng.lower_ap(in0), eng.lower_ap(in1)],
        outs=[eng.lower_ap(out)]))


def _build():
    nc = bacc_mod.Bacc("TRN2", target_bir_lowering=False, debug=False,
                       num_devices=N_CORES)
    img = nc.dram_tensor("img", [NIMG, C, H, W], F32, kind="ExternalInput")
    out = nc.dram_tensor("out", [NIMG, 1, H, W], BF16, kind="ExternalOutput")

    with TileContext(nc) as tc:
        with tc.tile_pool(name="const", bufs=1) as cpool, \
             tc.tile_pool(name="cin", bufs=4) as chpool, \
             tc.tile_pool(name="work", bufs=2) as wpool, \
             tc.tile_pool(name="psum", bufs=2, space="PSUM") as ppool:

            # ---- constants ----
            wmats = {}
            for d in (1, 2, 4, 7):
                wm = cpool.tile([128, 128], BF16, tag=f"wm{d}")
                nc.gpsimd.memset(wm[:], 1.0)
                # lhsT W[k, m] = 1 iff m == k - d  => out[m] = in[m + d]
                nc.gpsimd.affine_select(
                    out=wm[:], in_=wm[:],
                    compare_op=mybir.AluOpType.is_equal, fill=0.0,
                    base=d, channel_multiplier=-1, pattern=[[1, 128]])
                wmats[d] = wm

            # ---- per-block fresh-row loads (emitted 2 blocks ahead) ----
            # block b's A tile holds cmin of abs rows [r0-7, r0+rout+7);
            # rows [r0-7, r0+7) come from the previous A (SBUF copy),
            # fresh loads cover [r0+7, r0+rout+7) -> ct partitions
            # [14, 14+n). Block 0 loads [0, 121) -> partitions [7, 128).
            def emit_loads(b):
                r0 = b * RBLK
                rout = min(RBLK, H - r0)
                if b == 0:
                    src_lo, src_hi, dst0 = 0, r0 + rout + 7, 7
                else:
                    src_lo = r0 + 7
                    src_hi = min(r0 + rout + 7, H)
                    dst0 = HALO
                n = src_hi - src_lo
                ct = chpool.tile([128, C, NIMG, W], BF16, tag="ct")
                for c in range(C):
                    for i in range(NIMG):
                        nc.gpsimd.dma_start(
                            out=ct[dst0:dst0 + n, c, i, :],
                            in_=img[i, c, src_lo:src_hi, :])
                return ct

            pending = [emit_loads(0), emit_loads(1), emit_loads(2)]
            prevA = None
            for b in range(NBLK):
                r0 = b * RBLK
                rout = min(RBLK, H - r0)
                lo, hi = r0 - 7, r0 + rout + 7
                nvalid = min(hi, H) - lo   # valid A rows (rest +inf)
                ct = pending.pop(0)
                if b + 3 < NBLK:
                    pending.append(emit_loads(b + 3))

                # ---- channel min into padded A ----
                A = wpool.tile([128, NIMG, IPH], BF16, tag="A")
                nc.gpsimd.memset(A[:, :, 0:LPAD], BIG)
                nc.gpsimd.memset(A[:, :, LPAD + W:LPAD + W + 16], BIG)
                if hi > H:
                    # bottom edge: pre-fill tail rows with +inf, restrict
                    # ch-min to loaded rows (partition start stays 0)
                    nc.gpsimd.memset(A[96:128, :, :], BIG)
                    Amid = A[0:nvalid, :, LPAD:LPAD + W]
                    ctv = ct[0:nvalid]
                else:
                    Amid = A[:, :, LPAD:LPAD + W]
                    ctv = ct
                _tt(nc.vector, Amid, ctv[:, 0], ctv[:, 1])
                _tt(nc.vector, Amid, Amid, ctv[:, 2])
                if b == 0:
                    # top edge: overwrite garbage halo rows with +inf
                    nc.gpsimd.memset(A[0:7, :, :], BIG)
                else:
                    # halo: previous block's last 14 cmin rows
                    nc.sync.dma_start(out=A[0:HALO, :, :],
                                      in_=prevA[RBLK:RBLK + HALO, :, :])
                prevA = A

                # ---- h-pass: 4-step doubling, free-dim shifts ----
                Bv = wpool.tile([128, NIMG, IPH], BF16, tag="B")
                Cv = wpool.tile([128, NIMG, IPH], BF16, tag="C")
                cur = wpool.tile([128, NIMG, W], BF16, tag="cur")
                # s1: B[j] = min(A[j], A[j+1])          j in [0,1040)
                _tt(nc.vector, Bv[:, :, 0:1040], A[:, :, 0:1040],
                    A[:, :, 1:1041])
                # s2: C[j] = min(B[j], B[j+2])          j in [0,1038)
                _tt(nc.vector, Cv[:, :, 0:1038], Bv[:, :, 0:1038],
                    Bv[:, :, 2:1040])
                # s3: B[j] = min(C[j], C[j+4])          j in [0,1034)
                _tt(nc.vector, Bv[:, :, 0:1034], Cv[:, :, 0:1034],
                    Cv[:, :, 4:1038])
                # s4: cur[w] = min(B[w+1], B[w+8])  = min A[w-7 .. w+7]
                _tt(nc.vector, cur[:, :, :], Bv[:, :, 1:1025],
                    Bv[:, :, 8:1032])

                # ---- v-pass: PE shift matmuls; ACT copy; DVE min ----
                for d in (1, 2, 4, 7):
                    ps = ppool.tile([128, NIMG, 1024], F32, tag="ps")
                    for i in range(NIMG):
                        for h2 in range(2):
                            nc.tensor.matmul(
                                ps[:, i, h2 * 512:(h2 + 1) * 512],
                                wmats[d][:],
                                cur[:, i, h2 * 512:(h2 + 1) * 512],
                                start=True, stop=True)
                    sb = wpool.tile([128, NIMG, W], BF16, tag=f"sh{d}")
                    nc.scalar.copy(out=sb[:, :, :], in_=ps[:, :, :])
                    _tt(nc.vector, cur[:, :, :], cur[:, :, :], sb[:, :, :])

                for i in range(NIMG):
                    nc.sync.dma_start(out=out[i, 0, r0:r0 + rout, :],
                                      in_=cur[0:rout, i, :])

    nc.compile()
    return nc


def kernel(img: np.ndarray) -> np.ndarray:
    assert img.shape == (16, 3, 1024, 1024) and img.dtype == np.float32
    if "nc" not in _cache:
        _cache["nc"] = _build()
    nc = _cache["nc"]
    in_maps = [{"img": np.ascontiguousarray(img[2 * k:2 * k + 2])}
               for k in range(N_CORES)]
    res = bass_utils.run_bass_kernel_spmd(
        nc, in_maps, core_ids=list(range(N_CORES)))
    outs = [np.asarray(r["out"]).astype(np.float32) for r in res.results]
    return np.concatenate(outs, axis=0)
